# revision 36
# baseline (speedup 1.0000x reference)
"""AttentionPooling Trainium2 kernel (8-core data-parallel SPMD).

Reference computation per batch b (B=2048, T=200, E=H=64):
    att_in = [q, k, q-k, q*k]            (T, 4E)
    h   = elu(att_in @ W1 + b1)          (T, H)
    s   = h @ W2 + b2                    (T,)
    s   = where(mask, s, PAD); p = softmax(s)
    out = p @ k                          (E,)

Algebraic restructuring used here:
  att_in @ W1 = q@(W1a+W1c) + k@(W1b-W1c) + (q*k)@W1d
              = c(q)  +  k @ Wk  +  (q*k) @ Wp          [c is per-batch row]
  elu(x)+1 = relu(x) + min(exp(x), 1)   (exact), and softmax is invariant
  to additive constants, so  s ~ W2'relu(z) + W2'min(exp(z),1)  with
  z = k@Wk + (q*k)@Wp + c.  b2 and the +1 terms drop out of the softmax.
  Softmax uses no max-shift (scores are O(5) for this data distribution;
  exp stays comfortably finite in fp32) and folds masking in post-exp:
  p = (exp(s)*m) / sum(exp(s)*m).

Device layout: batches processed in pairs (2 batches span the 128
partitions: partition = 64*pb + e).  16 pairs form a group of 32 batches.
Host pre-packs keys twice (transposed bf16 for the score branch, natural
bf16 for the weighted sum) so no on-chip transpose of the big tensor is
needed.  Scores for all 16 pairs of a group accumulate into one [32, 200]
PSUM tile via zero-padded replicated-W2 stationaries.
"""

import os
import sys

import numpy as np

sys.path.insert(0, "/opt/trn_rl_repo")

import ml_dtypes

B, T, E, H = 2048, 200, 64, 64
NCORES = 8
BC = B // NCORES  # 256 batches per core
NPG = 16          # pairs per group
GB = 2 * NPG      # 32 batches per group
G = BC // GB      # 8 groups per core

BF16 = ml_dtypes.bfloat16

_PROGRAM_CACHE = {}


def _build_program():
    import concourse.bass as bass
    import concourse.tile as tile
    from concourse import bacc, mybir

    f32 = mybir.dt.float32
    bf16 = mybir.dt.bfloat16
    AX = mybir.AxisListType
    OP = mybir.AluOpType
    AF = mybir.ActivationFunctionType

    nc = bacc.Bacc("TRN2", debug=False)

    kT_d = nc.dram_tensor("kT", [G, 128, NPG * T], bf16, kind="ExternalInput")
    kn0_d = nc.dram_tensor("kn0", [G, 128, NPG * 128], bf16, kind="ExternalInput")
    kn1_d = nc.dram_tensor("kn1", [G, 72, NPG * 128], bf16, kind="ExternalInput")
    q_d = nc.dram_tensor("qp", [G, 128, NPG], f32, kind="ExternalInput")
    crow_d = nc.dram_tensor("crow", [G, 2, (NPG // 2) * 128], bf16, kind="ExternalInput")
    m01_d = nc.dram_tensor("m01", [G, GB, T], f32, kind="ExternalInput")
    # packed constants: one bf16 image [bdwk|bdwp|w2rep|ones] and one f32
    # image [id32|id64] — 2 DMAs instead of 6 at kernel start
    cbf_d = nc.dram_tensor("cbf", [128, 128 + 128 + NPG * GB + 2 * T], bf16,
                           kind="ExternalInput")
    cf32_d = nc.dram_tensor("cf32", [128, 96], f32, kind="ExternalInput")
    out_d = nc.dram_tensor("outp", [G, GB, E], f32, kind="ExternalOutput")

    with tile.TileContext(nc) as tc:
        with (
            tc.tile_pool(name="const", bufs=1) as cp,
            tc.tile_pool(name="gload", bufs=3) as gp,
            tc.tile_pool(name="qk", bufs=8) as qkp,
            tc.tile_pool(name="acts", bufs=4) as ap_,
            tc.tile_pool(name="sm", bufs=3) as smp,
            tc.tile_pool(name="zps", bufs=3, space=bass.MemorySpace.PSUM) as zp,
            tc.tile_pool(name="sps", bufs=2, space=bass.MemorySpace.PSUM) as sp,
        ):
            cbf = cp.tile([128, 128 + 128 + NPG * GB + 2 * T], bf16)
            nc.sync.dma_start(cbf[:], cbf_d[:])
            cf32 = cp.tile([128, 96], f32)
            nc.sync.dma_start(cf32[:], cf32_d[:])
            bdwk = cbf[:, 0:128]
            bdwp = cbf[:, 128:256]
            w2rep = cbf[:, 256:256 + NPG * GB]
            ones_r = cbf[0:2, 256 + NPG * GB:256 + NPG * GB + 2 * T]
            id32 = cf32[0:32, 0:32]
            id64 = cf32[0:64, 32:96]

            # per-group state carried across the software pipeline
            gstate = {}

            def emit_dma(g):
                # ordered by first use: qg/kTg feed the first qk+matmuls,
                # kn0/kn1 and m01 are tail-only
                qg = gp.tile([128, NPG], f32, tag="qg")
                nc.sync.dma_start(qg[:], q_d[g])
                kTg = gp.tile([128, NPG * T], bf16, tag="kTg")
                if g == 0:
                    # split so block-0 compute starts after the first half
                    nc.sync.dma_start(kTg[:, 0:8 * T], kT_d[g][:, 0:8 * T])
                    nc.sync.dma_start(kTg[:, 8 * T:], kT_d[g][:, 8 * T:])
                else:
                    nc.sync.dma_start(kTg[:], kT_d[g])
                crowg = gp.tile([2, (NPG // 2) * 128], bf16, tag="crowg")
                nc.sync.dma_start(crowg[:], crow_d[g])
                kn0g = gp.tile([128, NPG * 128], bf16, tag="kn0g")
                nc.sync.dma_start(kn0g[:], kn0_d[g])
                kn1g = gp.tile([72, NPG * 128], bf16, tag="kn1g")
                nc.sync.dma_start(kn1g[:], kn1_d[g])
                m01g = gp.tile([GB, T], f32, tag="m01g")
                nc.sync.dma_start(m01g[:], m01_d[g])
                gstate[g] = dict(kTg=kTg, kn0g=kn0g, kn1g=kn1g, qg=qg,
                                 crowg=crowg, m01g=m01g)

            def emit_block_head(g, jj):
                # one block = 4 pairs = two 2-pair duos at zsup cols 0 / 512
                st = gstate[g]
                zsup = zp.tile([128, 1024], f32, tag="z")
                for d2 in range(2):
                    j0 = 4 * jj + 2 * d2          # first pair of the duo
                    ksl = st["kTg"][:, j0 * T:(j0 + 2) * T]        # [128, 400]
                    qk = qkp.tile([128, 2 * T], bf16, tag="qk")
                    nc.gpsimd.tensor_scalar_mul(
                        qk[:, 0:T], st["kTg"][:, j0 * T:(j0 + 1) * T],
                        st["qg"][:, j0:j0 + 1])
                    nc.gpsimd.tensor_scalar_mul(
                        qk[:, T:2 * T], st["kTg"][:, (j0 + 1) * T:(j0 + 2) * T],
                        st["qg"][:, j0 + 1:j0 + 2])
                    zsl = zsup[:, 512 * d2: 512 * d2 + 2 * T]
                    nc.tensor.matmul(zsl, bdwk[:], ksl, start=True, stop=False)
                    nc.tensor.matmul(zsl, bdwp[:], qk[:], start=False, stop=False)
                    nc.tensor.matmul(
                        zsl, st["crowg"][0:2, (j0 // 2) * 128:(j0 // 2 + 1) * 128],
                        ones_r[:], start=False, stop=True,
                    )
                zv = zsup[:].rearrange("p (h c) -> p h c", h=2)[:, :, 0:2 * T]
                xsup = ap_.tile([128, 4 * T], f32, tag="x")
                xv = xsup[:].rearrange("p (h c) -> p h c", h=2)
                nc.scalar.activation(xv, zv, AF.Exp)
                # elu(z)+1 == max(z,0) + min(exp(z),1) exactly; two DVE ops
                xmsup = ap_.tile([128, 4 * T], bf16, tag="xm")
                nc.vector.tensor_scalar_min(xmsup[:], xsup[:], 1.0)
                xmv = xmsup[:].rearrange("p (h c) -> p h c", h=2)
                usup = ap_.tile([128, 4 * T], bf16, tag="ux")
                uv = usup[:].rearrange("p (h c) -> p h c", h=2)
                nc.vector.scalar_tensor_tensor(
                    uv, zv, 0.0, xmv, op0=OP.max, op1=OP.add)
                st[("blk", jj)] = usup

            def emit_block_mm3(g, jj):
                st = gstate[g]
                usup = st.pop(("blk", jj))
                if "tail" not in st:
                    # one tail PSUM bank holds scores/eT/o4/fin per group
                    tail = sp.tile([128, 512], f32, tag="tail")
                    st["tail"] = tail
                scores_ps = st["tail"][0:GB, 0:T]
                for j4 in range(4):
                    j = 4 * jj + j4
                    w2sl = w2rep[:, j * GB:(j + 1) * GB]
                    nc.tensor.matmul(
                        scores_ps, w2sl, usup[:, j4 * T:(j4 + 1) * T],
                        start=(j == 0), stop=(j == NPG - 1),
                        skip_group_check=True,
                    )

            def emit_tail_sm(g):
                # softmax numerators (no max shift) + row sums on ACT/DVE
                st = gstate[g]
                scores_ps = st["tail"][0:GB, 0:T]
                e_m = smp.tile([GB, T], f32, tag="em")
                nc.scalar.activation(e_m[:], scores_ps, AF.Exp)
                e_mm = smp.tile([GB, T], f32, tag="emm")
                nc.vector.tensor_mul(e_mm[:], e_m[:], st["m01g"][:])
                rs = smp.tile([GB, 1], f32, tag="rs")
                nc.vector.tensor_reduce(rs[:], e_mm[:], axis=AX.X, op=OP.add)
                ri = smp.tile([GB, 1], f32, tag="ri")
                nc.vector.reciprocal(ri[:], rs[:])
                st["e_mm"] = e_mm
                st["ri"] = ri

            def emit_tail_pe(g):
                st = gstate.pop(g)
                tail = st["tail"]
                e_mm, ri = st["e_mm"], st["ri"]
                eT0_ps = tail[:, 200:232]
                eT1_ps = tail[0:72, 232:264]
                o4 = tail[:, 264:296]
                fin_ps = tail[0:GB, 296:360]
                nc.tensor.transpose(eT0_ps, e_mm[:, 0:128], id32[:])
                nc.tensor.transpose(eT1_ps, e_mm[:, 128:200], id32[:])
                eT0 = smp.tile([128, 32], bf16, tag="eT0")
                nc.scalar.copy(eT0[:], eT0_ps)
                eT1 = smp.tile([72, 32], bf16, tag="eT1")
                nc.scalar.copy(eT1[:], eT1_ps)
                for j in range(NPG):
                    nc.tensor.matmul(
                        o4[:, 2 * j:2 * j + 2],
                        st["kn0g"][:, j * 128:(j + 1) * 128],
                        eT0[:, 2 * j:2 * j + 2], start=True, stop=False,
                        skip_group_check=True,
                    )
                    nc.tensor.matmul(
                        o4[:, 2 * j:2 * j + 2],
                        st["kn1g"][:, j * 128:(j + 1) * 128],
                        eT1[:, 2 * j:2 * j + 2], start=False, stop=True,
                        skip_group_check=True,
                    )
                osb = smp.tile([64, GB], f32, tag="osb")
                o4v = o4.rearrange("p (j two) -> p j two", two=2)
                osbv = osb[:].rearrange("p (j two) -> p j two", two=2)
                nc.scalar.copy(osbv[:, :, 0:1], o4v[0:64, :, 0:1])
                nc.scalar.copy(osbv[:, :, 1:2], o4v[64:128, :, 1:2])
                nc.tensor.transpose(fin_ps, osb[:], id64[:])
                fin = smp.tile([GB, 64], f32, tag="fins")
                nc.scalar.mul(fin[:], fin_ps, ri[:])
                nc.sync.dma_start(out_d[g], fin[:])

            # software pipeline: mm3 deferred one block; tail spans groups;
            # DMA prefetched two groups ahead
            emit_dma(0)
            emit_dma(1)
            for g in range(G):
                for jj in range(4):
                    emit_block_head(g, jj)
                    if jj == 0:
                        if g > 0:
                            emit_block_mm3(g - 1, 3)
                            emit_tail_sm(g - 1)
                    elif jj == 1:
                        if g > 0:
                            emit_tail_pe(g - 1)
                        emit_block_mm3(g, 0)
                    elif jj == 2:
                        if g + 2 < G:
                            emit_dma(g + 2)
                        emit_block_mm3(g, 1)
                    else:
                        emit_block_mm3(g, jj - 1)
            emit_block_mm3(G - 1, 3)
            emit_tail_sm(G - 1)
            emit_tail_pe(G - 1)

    nc.compile()
    return nc


def _pack_inputs(queries, keys, mask, W1, b1, W2, b2):
    """Host-side packing into per-core input maps."""
    queries = np.asarray(queries, dtype=np.float32)
    keys = np.asarray(keys, dtype=np.float32)
    mask = np.asarray(mask)
    W1 = np.asarray(W1, dtype=np.float32)
    b1 = np.asarray(b1, dtype=np.float32)
    W2 = np.asarray(W2, dtype=np.float32)

    Wq = W1[0:E] + W1[2 * E:3 * E]        # query block + diff block
    Wk = W1[E:2 * E] - W1[2 * E:3 * E]    # key block - diff block
    Wp = W1[3 * E:4 * E]                  # product block

    # per-batch bias row c = q @ Wq + b1   -> (B, H)
    cvals = queries[:, 0, :] @ Wq + b1[None, :]

    # keys reshaped [core, group, pair, pb, t, e]
    K6 = keys.reshape(NCORES, G, NPG, 2, T, E)
    kT = np.ascontiguousarray(K6.transpose(0, 1, 3, 5, 2, 4)).reshape(
        NCORES, G, 128, NPG * T).astype(BF16)
    kn = np.ascontiguousarray(K6.transpose(0, 1, 4, 2, 3, 5)).reshape(
        NCORES, G, T, NPG * 128).astype(BF16)
    kn0 = np.ascontiguousarray(kn[:, :, :128])
    kn1 = np.ascontiguousarray(kn[:, :, 128:])

    Q5 = queries[:, 0, :].reshape(NCORES, G, NPG, 2, E)
    qp = np.ascontiguousarray(Q5.transpose(0, 1, 3, 4, 2)).reshape(
        NCORES, G, 128, NPG).astype(np.float32)

    # duo layout: row r of crow[g] holds pair (2*jj2 + r)'s c-row at free
    # offset jj2*128
    crow = np.ascontiguousarray(
        cvals.reshape(NCORES, G, NPG // 2, 2, 128).transpose(0, 1, 3, 2, 4)
    ).reshape(NCORES, G, 2, (NPG // 2) * 128).astype(BF16)

    m01 = mask.reshape(NCORES, G, GB, T).astype(np.float32)

    bdwk = np.zeros((128, 128), np.float32)
    bdwk[0:64, 0:64] = Wk
    bdwk[64:128, 64:128] = Wk
    bdwp = np.zeros((128, 128), np.float32)
    bdwp[0:64, 0:64] = Wp
    bdwp[64:128, 64:128] = Wp

    w2rep = np.zeros((128, NPG * GB), np.float32)
    w2c = W2[:, 0]
    for j in range(NPG):
        w2rep[0:64, j * GB + 2 * j] = w2c
        w2rep[64:128, j * GB + 2 * j + 1] = w2c

    onesr = np.zeros((128, 2 * T), np.float32)
    onesr[0, 0:T] = 1.0
    onesr[1, T:2 * T] = 1.0
    cbf = np.concatenate(
        [bdwk, bdwp, w2rep, onesr], axis=1).astype(BF16)
    cf32 = np.zeros((128, 96), np.float32)
    cf32[0:32, 0:32] = np.eye(32)
    cf32[0:64, 32:96] = np.eye(64)
    consts = {"cbf": cbf, "cf32": cf32}

    in_maps = []
    for c in range(NCORES):
        m = {
            "kT": kT[c], "kn0": kn0[c], "kn1": kn1[c],
            "qp": qp[c], "crow": crow[c], "m01": m01[c],
        }
        m.update(consts)
        in_maps.append(m)
    return in_maps


def kernel(queries, keys, mask, W1, b1, W2, b2):
    from concourse import bass_utils

    key = "prog"
    if key not in _PROGRAM_CACHE:
        _PROGRAM_CACHE[key] = _build_program()
    nc = _PROGRAM_CACHE[key]

    in_maps = _pack_inputs(queries, keys, mask, W1, b1, W2, b2)
    res = bass_utils.run_bass_kernel_spmd(nc, in_maps, list(range(NCORES)))
    outs = [res.results[c]["outp"] for c in range(NCORES)]  # [G, GB, E] each
    out = np.stack(outs).reshape(B, E).astype(np.float32)
    return out[:, None, :]


if __name__ == "__main__":
    sys.path.insert(0, os.path.dirname(os.path.abspath(__file__)))
    import reference

    inputs = reference.setup_inputs()
    expected = np.asarray(reference.reference(**inputs))
    actual = kernel(**{k: np.asarray(v) for k, v in inputs.items()})
    err = np.abs(actual - expected).max()
    rel = err / max(1e-12, np.abs(expected).max())
    print("absmax err:", err, "rel:", rel)
